# revision 1
# baseline (speedup 1.0000x reference)
"""Causal self-attention block (LN -> QKV -> causal attention -> out-proj)
on 8 Trainium2 NeuronCores.

Sharding: core = 2*batch + head_group. Each core handles one batch element
(S=2048 tokens) and 8 of the 16 heads (tensor-parallel split of w_qkv along
the head axis and w_out along its input dim). The two partial outputs per
batch are summed on the host (the all-reduce of the sharding hint).

Device kernel layout strategy (per core):
  - LayerNorm in natural layout [s, d], then PE-transpose to xnT [d, s]
    (contraction dim must sit on partitions for matmuls).
  - QKV projection computes q^T/k^T in [head_dim, s] layout directly and V in
    natural [s, head_dim] layout, so causal attention needs no further
    transposes: scores are computed transposed, ST[k, q] = k . q, softmax'd
    along the partition-free axis via exp + a ones-column appended to V
    (the PV matmul then yields both y^T and the softmax row-sums).
  - ln_scale/ln_bias/b_qkv/softmax-scale/b_out are all folded into the
    weights on the host; matmuls run as float32r (FP22, full PE rate).
"""

import os

# the device path runs through jax's axon PJRT plugin; make sure a
# pre-set JAX_PLATFORMS doesn't hide it (unset = all plugins load)
_jp = os.environ.get("JAX_PLATFORMS")
if _jp and "axon" not in _jp:
    os.environ["JAX_PLATFORMS"] = f"axon,{_jp}"

import numpy as np

import concourse.bass as bass
import concourse.mybir as mybir
import concourse.tile as tile
from concourse import bacc
from concourse.bass_utils import run_bass_kernel_spmd
from concourse.masks import make_identity

B, S, D, H, HD = 4, 2048, 1024, 16, 64
HL = H // 2          # heads per core (local)
NCH = D // 128       # 8 contraction chunks
NSB = S // 128       # 16 s-blocks
NQS = S // 512       # 4 q-superblocks
NEG = -1.0e38
LN_EPS = 1e-6

f32 = mybir.dt.float32
f32r = mybir.dt.float32r

_cache = {}


def build_program():
    nc = bacc.Bacc()

    x_d = nc.declare_dram_parameter("x", [S, D], f32, isOutput=False)
    wqk_d = nc.declare_dram_parameter("wqk", [NCH, 128, 1024], f32r, isOutput=False)
    wv_d = nc.declare_dram_parameter("wv", [NCH, 128, 512], f32r, isOutput=False)
    bqk_d = nc.declare_dram_parameter("bqk", [128, 2, 4], f32, isOutput=False)
    bv1_d = nc.declare_dram_parameter("bv1", [1, 512], f32r, isOutput=False)
    vones_d = nc.declare_dram_parameter("vones", [1, 128], f32r, isOutput=False)
    wout_d = nc.declare_dram_parameter("wout", [4, 128, 1024], f32r, isOutput=False)
    out_d = nc.declare_dram_parameter("out", [S, D], f32, isOutput=True)

    with tile.TileContext(nc, pool_alloc_mode="queue") as tc:
        with (
            tc.tile_pool(name="singles", bufs=1) as singles,
            tc.tile_pool(name="qkT", bufs=1) as qkTp,
            tc.tile_pool(name="vpool", bufs=1) as vpool,
            tc.tile_pool(name="pscm", bufs=1, space="PSUM") as pscm,
        ):
            # ---- constants ----
            ident = singles.tile([128, 128], f32)
            make_identity(nc, ident)
            identb = singles.tile([128, 128], mybir.dt.bfloat16)
            make_identity(nc, identb)
            maskTb = singles.tile([128, 128], mybir.dt.bfloat16)
            nc.gpsimd.memset(maskTb, 0.0)
            nc.gpsimd.affine_select(
                out=maskTb, in_=maskTb,
                compare_op=mybir.AluOpType.is_ge,
                fill=NEG, base=0,
                pattern=[[1, 128]], channel_multiplier=-1,
            )
            eps_t = singles.tile([128, 1], f32)
            nc.vector.memset(eps_t, LN_EPS)
            bqk_t = singles.tile([128, 2, 4], f32)
            nc.sync.dma_start(out=bqk_t, in_=bqk_d[:, :, :])
            bv1_t = singles.tile([1, 512], f32r)
            nc.sync.dma_start(out=bv1_t, in_=bv1_d[:, :])
            vones_t = singles.tile([1, 128], f32r)
            nc.sync.dma_start(out=vones_t, in_=vones_d[:, :])

            # ---- persistent activations ----
            qT = qkTp.tile([128, 4, S], f32r)   # [pair-row, pair, s]
            kT = qkTp.tile([128, 4, S], f32r)
            # V'' [s-row, s-block, head, 65] (col 64 = ones)
            vpp = vpool.tile([128, NSB, HL, HD + 1], f32r)
            nc.gpsimd.memset(vpp[:, :, :, HD : HD + 1].bitcast(f32), 1.0)

            # ================= Phase A: LayerNorm + transpose =================
            with tc.tile_pool(name="xnT", bufs=1) as xnTp:
                xnT = xnTp.tile([128, NCH, S], f32r)
                with (
                    tc.tile_pool(name="atmp", bufs=5) as atmp,
                    tc.tile_pool(name="astat", bufs=8) as astat,
                ):
                    for i in range(NSB):
                        x_t = atmp.tile([128, D], f32, tag="x")
                        nc.sync.dma_start(out=x_t, in_=x_d[i * 128 : (i + 1) * 128, :])
                        stats = astat.tile([128, 2, 6], f32, tag="stats")
                        nc.vector.bn_stats(out=stats[:, 0, :], in_=x_t[:, 0:512])
                        nc.vector.bn_stats(out=stats[:, 1, :], in_=x_t[:, 512:1024])
                        mv = astat.tile([128, 2], f32, tag="mv")
                        nc.vector.bn_aggr(out=mv, in_=stats)
                        std_t = astat.tile([128, 1], f32, tag="std")
                        nc.scalar.activation(
                            out=std_t, in_=mv[:, 1:2],
                            func=mybir.ActivationFunctionType.Sqrt,
                            bias=eps_t, scale=1.0,
                        )
                        rstd_t = astat.tile([128, 1], f32, tag="rstd")
                        nc.vector.reciprocal(out=rstd_t, in_=std_t)
                        xn_t = atmp.tile([128, D], f32, tag="xn")
                        nc.vector.tensor_scalar(
                            out=xn_t, in0=x_t,
                            scalar1=mv[:, 0:1], scalar2=rstd_t,
                            op0=mybir.AluOpType.subtract, op1=mybir.AluOpType.mult,
                        )
                        for c4 in range(0, NCH, 4):
                            pst = pscm.tile([128, 4, 128], f32, tag="yt", bufs=4)
                            for c in range(c4, c4 + 4):
                                nc.tensor.transpose(
                                    pst[:, c - c4, :],
                                    xn_t[:, c * 128 : (c + 1) * 128],
                                    ident,
                                )
                            nc.scalar.activation(
                                out=xnT[:, c4 : c4 + 4, i * 128 : (i + 1) * 128],
                                in_=pst,
                                func=mybir.ActivationFunctionType.Copy,
                            )

                # ================= Phase B: QKV projection =================
                with (
                    tc.tile_pool(name="wqk", bufs=2) as wqkp,
                    tc.tile_pool(name="wvp", bufs=1) as wvp,
                ):
                    def emit_qk(t, p):
                        fb = t * 4 + p
                        w_t = wqkp.tile([128, NCH, 128], f32r, tag="wqk",
                                        name=f"wqk_{t}_{p}")
                        nc.sync.dma_start(
                            out=w_t,
                            in_=wqk_d[:, :, fb * 128 : (fb + 1) * 128].rearrange(
                                "c d f -> d c f"
                            ),
                        )
                        dest = qT if t == 0 else kT
                        for sb in range(NQS):
                            ps = pscm.tile([128, 512], f32, tag="st", bufs=2,
                                           name=f"psqk_{t}_{p}_{sb}")
                            for c in range(NCH):
                                nc.tensor.matmul(
                                    ps,
                                    w_t[:, c, :],
                                    xnT[:, c, sb * 512 : (sb + 1) * 512],
                                    start=(c == 0),
                                    stop=(c == NCH - 1),
                                )
                            nc.vector.tensor_scalar_add(
                                out=dest[:, p, sb * 512 : (sb + 1) * 512],
                                in0=ps,
                                scalar1=bqk_t[:, t, p : p + 1],
                            )

                    def emit_v():
                        wv_t = wvp.tile([128, NCH, 512], f32r)
                        for c in range(NCH):
                            nc.sync.dma_start(out=wv_t[:, c, :], in_=wv_d[c, :, :])
                        for i in range(NSB):
                            psv = pscm.tile([128, 512], f32, tag="st", bufs=2,
                                            name=f"psv_{i}")
                            for c in range(NCH):
                                nc.tensor.matmul(
                                    psv,
                                    xnT[:, c, i * 128 : (i + 1) * 128],
                                    wv_t[:, c, :],
                                    start=(c == 0),
                                    stop=False,
                                )
                            # += ones[s] x bv  (rank-1 bias update)
                            nc.tensor.matmul(
                                psv, vones_t, bv1_t, start=False, stop=True,
                            )
                            nc.vector.tensor_copy(
                                vpp[:, i, :, 0:HD],
                                psv.rearrange("p (h v) -> p h v", v=HD),
                            )

                    # pair 0 first, then V, so attention on heads 0/1 can
                    # start while the rest of the projection still runs
                    emit_qk(0, 0)
                    emit_qk(1, 0)
                    emit_v()
                    for p in range(1, 4):
                        emit_qk(0, p)
                        emit_qk(1, p)

            # ================= Phase C: causal attention =================
            with tc.tile_pool(name="ytall", bufs=1) as ytallp:
                ytall = ytallp.tile([128, 4, S], f32r)  # [pair-row, pair, s]
                with (
                    tc.tile_pool(name="ptp", bufs=6) as ptp,
                    tc.tile_pool(name="ctmp", bufs=4) as ctmp,
                    tc.tile_pool(name="dscr", bufs=8, space="DRAM") as dscr,
                    tc.tile_pool(name="woutp", bufs=1) as woutp,
                    tc.tile_pool(name="ypool", bufs=3) as ypool,
                ):
                    wout_t = woutp.tile([128, 4, 1024], f32r)
                    for c in range(4):
                        nc.sync.dma_start(out=wout_t[:, c, :], in_=wout_d[c, :, :])
                    def emit_outproj(i):
                        y_t = ypool.tile([128, 1024], f32, tag="y",
                                         name=f"y_{i}")
                        for nh in range(2):
                            # alternate tags: the yt slots are idle during
                            # the output projection, use them for depth
                            pso = pscm.tile([128, 512], f32,
                                            tag=("st" if nh == 0 else "yt"),
                                            bufs=(2 if nh == 0 else 4),
                                            name=f"pso_{i}_{nh}")
                            for c in range(4):
                                nc.tensor.matmul(
                                    pso,
                                    ytall[:, c, i * 128 : (i + 1) * 128],
                                    wout_t[:, c, nh * 512 : (nh + 1) * 512],
                                    start=(c == 0),
                                    stop=(c == 3),
                                )
                            nc.vector.tensor_copy(
                                y_t[:, nh * 512 : (nh + 1) * 512], pso
                            )
                        nc.sync.dma_start(
                            out=out_d[i * 128 : (i + 1) * 128, :], in_=y_t
                        )

                    for sb in range(NQS):
                        for p in range(4):
                            # the pair's two heads (PE rows 0:64 / 64:128)
                            # run as adjacent matmuls -> concurrent row-groups
                            q0 = sb * 512
                            jmax = 4 * sb + 3
                            yts = [
                                pscm.tile([HD + 1, 512], f32, tag="yt",
                                          bufs=4, name=f"yt_{2 * p + hf}_{sb}")
                                for hf in range(2)
                            ]
                            for j in range(jmax + 1):
                                r = max(0, j - 4 * sb)
                                diag = j >= 4 * sb
                                L = 512 - 128 * r
                                st = pscm.tile([128, 1024], f32, tag="st",
                                               bufs=2, name=f"st_{p}_{sb}_{j}")
                                pt = ptp.tile([128, 1024], f32r, tag="pt")
                                for hf in range(2):
                                    rows = slice(hf * HD, (hf + 1) * HD)
                                    # hf0 packs left in bank 0; hf1 must stay
                                    # bank-aligned at 512 (matmul outputs
                                    # cannot cross a PSUM bank boundary)
                                    lo = hf * 512
                                    nc.tensor.matmul(
                                        st[:, lo : lo + L],
                                        kT[rows, p, j * 128 : (j + 1) * 128],
                                        qT[rows, p, q0 + r * 128 : q0 + 512],
                                        start=True, stop=not diag,
                                    )
                                if diag:
                                    # causal mask folded in on the PE:
                                    # st[diag] += I.T @ maskT
                                    for hf in range(2):
                                        nc.tensor.matmul(
                                            st[:, hf * 512 : hf * 512 + 128],
                                            identb,
                                            maskTb,
                                            start=False, stop=True,
                                        )
                                # one wide exp across both heads (for r>0 the
                                # [L:512) strip is unread garbage)
                                nc.scalar.activation(
                                    out=pt[:, 0 : 512 + L],
                                    in_=st[:, 0 : 512 + L],
                                    func=mybir.ActivationFunctionType.Exp,
                                )
                                for hf in range(2):
                                    nc.tensor.matmul(
                                        yts[hf][:, r * 128 : 512],
                                        vpp[:, j, 2 * p + hf, :],
                                        pt[:, hf * 512 : hf * 512 + L],
                                        start=(j == 0),
                                        stop=(j == jmax),
                                    )
                            # per-superblock softmax normalization epilogue
                            for hf in range(2):
                                rows = slice(hf * HD, (hf + 1) * HD)
                                yt = yts[hf]
                                ssum = ctmp.tile([1, 512], f32, tag="ssum")
                                nc.vector.tensor_copy(ssum, yt[HD : HD + 1, :])
                                dsum = dscr.tile([512], f32, tag="dsum")
                                nc.sync.dma_start(out=dsum, in_=ssum)
                                sums4 = ctmp.tile([4, 128], f32, tag="sums4")
                                nc.sync.dma_start(
                                    out=sums4,
                                    in_=dsum.rearrange("(a b) -> a b", b=128),
                                )
                                sinv4 = ctmp.tile([4, 128], f32, tag="sinv4")
                                nc.vector.reciprocal(out=sinv4, in_=sums4)
                                dsinv = dscr.tile([512], f32, tag="dsinv")
                                nc.sync.dma_start(
                                    out=dsinv.rearrange("(a b) -> a b", b=128),
                                    in_=sinv4,
                                )
                                src = dsinv[:]
                                bcast = bass.AP(
                                    tensor=src.tensor,
                                    offset=src.offset,
                                    ap=[[0, HD]] + list(src.ap),
                                )
                                binv = ctmp.tile([HD, 512], f32, tag="binv")
                                nc.sync.dma_start(out=binv, in_=bcast)
                                nc.vector.tensor_mul(
                                    out=ytall[rows, p, q0 : q0 + 512],
                                    in0=yt[0:HD, :],
                                    in1=binv,
                                )

                    for i in range(NSB):
                        emit_outproj(i)

    nc.finalize()
    return nc


def _prep_core_inputs(x, ln_scale, ln_bias, w_qkv, b_qkv, w_out):
    """Host-side folding + per-core input maps."""
    scale = np.float32(HD ** -0.5)
    # qkv = xn@W + b_qkv, xn = z*ln_scale + ln_bias  =>  z @ (ln_scale*W) + (ln_bias@W + b_qkv)
    b_eff = b_qkv + np.einsum(
        "d,dhf->hf", ln_bias.astype(np.float64), w_qkv.astype(np.float64)
    ).astype(np.float32)
    w_eff = ln_scale[:, None, None] * w_qkv
    wq = w_eff[:, :, 0:64] * scale
    wk = w_eff[:, :, 64:128]
    wv = w_eff[:, :, 128:192]
    bq = b_eff[:, 0:64] * scale
    bk = b_eff[:, 64:128]
    bv = b_eff[:, 128:192]

    in_maps = []
    for core in range(8):
        b, g = core // 2, core % 2
        hsel = slice(g * HL, (g + 1) * HL)
        # [D, 4 pairs, 128] with head 2p in rows 0:64, head 2p+1 in 64:128
        qp = wq[:, hsel].reshape(D, 4, 128)
        kp = wk[:, hsel].reshape(D, 4, 128)
        wqk = np.concatenate(
            [qp.reshape(D, 512), kp.reshape(D, 512)], axis=1
        ).reshape(NCH, 128, 1024)
        wv_g = np.ascontiguousarray(wv[:, hsel].reshape(D, 512)).reshape(
            NCH, 128, 512
        )
        bq_p = bq[hsel].reshape(4, 128)
        bk_p = bk[hsel].reshape(4, 128)
        bqk = np.ascontiguousarray(
            np.stack([bq_p, bk_p], axis=0).transpose(2, 0, 1)
        )
        bv1 = np.ascontiguousarray(bv[hsel].reshape(1, 512))
        wout = np.ascontiguousarray(
            w_out[g * 512 : (g + 1) * 512, :].reshape(4, 128, 1024)
        )
        in_maps.append(
            {
                "x": np.ascontiguousarray(x[b]),
                "wqk": np.ascontiguousarray(wqk),
                "wv": wv_g,
                "bqk": bqk,
                "bv1": bv1,
                "vones": np.ones((1, 128), np.float32),
                "wout": wout,
            }
        )
    return in_maps


def kernel(x, mask, ln_scale, ln_bias, w_qkv, b_qkv, w_out, b_out, **run_kwargs):
    x = np.asarray(x, np.float32)
    ln_scale = np.asarray(ln_scale, np.float32)
    ln_bias = np.asarray(ln_bias, np.float32)
    w_qkv = np.asarray(w_qkv, np.float32)
    b_qkv = np.asarray(b_qkv, np.float32)
    w_out = np.asarray(w_out, np.float32)
    b_out = np.asarray(b_out, np.float32)
    if "nc" not in _cache:
        _cache["nc"] = build_program()
    nc = _cache["nc"]
    in_maps = _prep_core_inputs(x, ln_scale, ln_bias, w_qkv, b_qkv, w_out)
    res = run_bass_kernel_spmd(nc, in_maps, list(range(8)), **run_kwargs)
    _cache["last_result"] = res
    out = np.empty((B, S, D), np.float32)
    for b in range(B):
        out[b] = res.results[2 * b]["out"] + res.results[2 * b + 1]["out"]
    out += np.asarray(b_out)[None, None, :]
    return out



# revision 31
# speedup vs baseline: 1.1599x; 1.1599x over previous
"""Causal self-attention block (LN -> QKV -> causal attention -> out-proj)
on 8 Trainium2 NeuronCores.

Sharding: core = 2*batch + head_group. Each core handles one batch element
(S=2048 tokens) and 8 of the 16 heads (tensor-parallel split of w_qkv along
the head axis and w_out along its input dim). The two partial outputs per
batch are summed on the host (the all-reduce of the sharding hint).

Device kernel strategy (per core):
  - All matmuls run bf16 (1 cycle/row at any N; f32r pays 4x below N=256),
    except the PV attention matmuls which run fp8e4 in DoubleRow perf mode
    (K=256 across a pair of key tiles, 0.5 cycles/row). Softmax weights pt
    are raw fp8 (exp writes fp8 directly); V is split into fp8 hi + fp8 lo
    (error feedback) so PV accuracy stays near bf16. Softmax denominators
    come from a DoubleRow ones matmul into a separate PSUM row.
  - Causal masking is done on GpSimd (affine_select / memset zeroing the
    fp8 pt tiles after exp), keeping the PE free of mask matmuls.
  - LayerNorm: stats on DVE (bn_stats/bn_aggr), sqrt on Act, normalize on
    GpSimd; x^T produced by PE transposes in bf16.
  - Single merged pipeline: attention superblock sb needs only QK(<=sb) and
    V-blocks(<=sb), so LN/transpose/V/QK/out-proj work items are interleaved
    between attention score/PV pairs to keep the PE busy while the Act
    engine works through the exp stream (the attention-phase bottleneck).
"""

import os
from collections import deque

# the device path runs through jax's axon PJRT plugin; make sure a
# pre-set JAX_PLATFORMS doesn't hide it (unset = all plugins load)
_jp = os.environ.get("JAX_PLATFORMS")
if _jp and "axon" not in _jp:
    os.environ["JAX_PLATFORMS"] = f"axon,{_jp}"

import numpy as np
import ml_dtypes

import concourse.bass as bass
import concourse.mybir as mybir
import concourse.tile as tile
from concourse import bacc
from concourse.bass_utils import run_bass_kernel_spmd
from concourse.masks import make_identity

B, S, D, H, HD = 4, 2048, 1024, 16, 64
HL = H // 2          # heads per core (local)
NCH = D // 128       # 8 contraction chunks
NSB = S // 128       # 16 s-blocks
NQS = S // 512       # 4 q-superblocks
LN_EPS = 1e-6

f32 = mybir.dt.float32
bf16 = mybir.dt.bfloat16
f8 = mybir.dt.float8e4
DR = mybir.MatmulPerfMode.DoubleRow

_cache = {}


def build_program(has_qk_bias, has_v_bias):
    nc = bacc.Bacc()

    x_d = nc.declare_dram_parameter("x", [S, D], f32, isOutput=False)
    wqk_d = nc.declare_dram_parameter("wqk", [128, NCH, 1024], bf16, isOutput=False)
    wv_d = nc.declare_dram_parameter("wv", [128, NCH, 512], bf16, isOutput=False)
    wout_d = nc.declare_dram_parameter("wout", [128, 4, 1024], bf16, isOutput=False)
    out_d = nc.declare_dram_parameter("out", [S, D], f32, isOutput=True)
    dbg = bool(int(os.environ.get("KDBG_DUMP", "0")))
    if dbg:
        ytall_d = nc.declare_dram_parameter("ytall", [128, 4, S], mybir.dt.bfloat16, isOutput=True)
        qT_d = nc.declare_dram_parameter("qTd", [128, 4, S], mybir.dt.bfloat16, isOutput=True)
        kT_d = nc.declare_dram_parameter("kTd", [128, 4, S], mybir.dt.bfloat16, isOutput=True)
        vh_d = nc.declare_dram_parameter("vhd", [128, NSB, HL, HD], mybir.dt.float8e4, isOutput=True)
        vl_d = nc.declare_dram_parameter("vld", [128, NSB, HL, HD], mybir.dt.float8e4, isOutput=True)
    if has_qk_bias:
        bqk_d = nc.declare_dram_parameter("bqk", [128, 2, 4], f32, isOutput=False)
    if has_v_bias:
        vones_d = nc.declare_dram_parameter("vones", [1, 128], bf16, isOutput=False)
        bv1_d = nc.declare_dram_parameter("bv1", [1, 512], bf16, isOutput=False)

    with tile.TileContext(nc, pool_alloc_mode="queue") as tc:
        with (
            tc.tile_pool(name="singles", bufs=1) as singles,
            tc.tile_pool(name="persist", bufs=1) as persist,
            tc.tile_pool(name="xpool", bufs=4) as xpool,
            tc.tile_pool(name="xnpool", bufs=3) as xnpool,
            tc.tile_pool(name="stat", bufs=8) as statp,
            tc.tile_pool(name="ptp", bufs=6) as ptp,
            tc.tile_pool(name="ptbp", bufs=4) as ptbp,
            tc.tile_pool(name="sip", bufs=4) as sip,
            tc.tile_pool(name="bip", bufs=4) as bip,
            tc.tile_pool(name="ypool", bufs=2) as ypool,
            tc.tile_pool(name="dscr", bufs=8, space="DRAM") as dscr,
            tc.tile_pool(name="projps", bufs=2, space="PSUM") as projps,
            tc.tile_pool(name="stps", bufs=2, space="PSUM") as stps,
            tc.tile_pool(name="ytps", bufs=1, space="PSUM") as ytps,
            tc.tile_pool(name="smps", bufs=1, space="PSUM") as smps,
        ):
            # ---- constants ----
            ones64 = None
            identb = singles.tile([128, 128], bf16)
            make_identity(nc, identb)
            eps_t = singles.tile([128, 1], f32)
            nc.vector.memset(eps_t, LN_EPS)
            negc_t = singles.tile([128, 1], f32)
            nc.vector.memset(negc_t, -2.0)
            ones64f = singles.tile([1, 64], f32)
            nc.vector.memset(ones64f, 1.0)
            ones64 = singles.tile([1, 64], mybir.dt.float32r)
            with nc.allow_low_precision(reason="fp22 ones row"):
                nc.scalar.activation(
                    out=ones64, in_=ones64f,
                    func=mybir.ActivationFunctionType.Copy, scale=1.0,
                )
            ones8 = singles.tile([128, 2, 16], f8)
            nc.gpsimd.memset(ones8, 1.0)
            ones_bf = singles.tile([128, 16], bf16)
            nc.gpsimd.memset(ones_bf, 1.0)
            if has_qk_bias:
                bqk_t = singles.tile([128, 2, 4], f32)
                nc.sync.dma_start(out=bqk_t, in_=bqk_d[:, :, :])
            if has_v_bias:
                vones_t = singles.tile([1, 128], bf16)
                nc.sync.dma_start(out=vones_t, in_=vones_d[:, :])
                bv1_t = singles.tile([1, 512], bf16)
                nc.sync.dma_start(out=bv1_t, in_=bv1_d[:, :])

            # ---- weights (tiles now, DMAs deferred past the x DMAs) ----
            wv_t = singles.tile([128, NCH, 512], bf16)
            wqk_t = singles.tile([128, NCH, 1024], bf16)
            wout_t = singles.tile([128, 4, 1024], bf16)

            def wv_t_dma():
                nc.sync.dma_start(out=wv_t, in_=wv_d[:, :, :])

            def wqk_t_dma():
                for fb in range(8):
                    nc.sync.dma_start(
                        out=wqk_t[:, :, fb * 128 : (fb + 1) * 128],
                        in_=wqk_d[:, :, fb * 128 : (fb + 1) * 128],
                    )

            def wout_t_dma():
                nc.sync.dma_start(out=wout_t, in_=wout_d[:, :, :])

            # ---- persistent activations ----
            xnT = persist.tile([128, NCH, S], bf16)
            qT = persist.tile([128, 4, S], bf16)   # [pair-row, pair, s]
            kT = persist.tile([128, 4, S], bf16)
            vh = persist.tile([128, NSB, HL, HD], f8)   # V hi fp8
            vl = persist.tile([128, NSB, HL, HD], f8)   # V residual fp8
            # bf16 V for the first 4 key blocks: sb=0's PV runs in bf16
            # (fp8 softmax weights are too noisy for the few-key early rows)
            vb16 = persist.tile([128, 4, HL, HD], bf16)
            ytall = persist.tile([128, 4, S], bf16)

            xns = {}

            # ================= proj work items =================
            def ln_item(i):
                def f():
                    x_t = xpool.tile([128, D], f32, tag="x", name=f"x_{i}")
                    stats = statp.tile([128, 2, 6], f32, tag="stats")
                    for hh in range(2):
                        nc.sync.dma_start(
                            out=x_t[:, hh * 512 : (hh + 1) * 512],
                            in_=x_d[i * 128 : (i + 1) * 128,
                                    hh * 512 : (hh + 1) * 512],
                        )
                        nc.vector.bn_stats(
                            out=stats[:, hh, :],
                            in_=x_t[:, hh * 512 : (hh + 1) * 512],
                        )
                    mv = statp.tile([128, 2], f32, tag="mv")
                    nc.vector.bn_aggr(out=mv, in_=stats)
                    std_t = statp.tile([128, 1], f32, tag="std")
                    nc.scalar.activation(
                        out=std_t, in_=mv[:, 1:2],
                        func=mybir.ActivationFunctionType.Sqrt,
                        bias=eps_t, scale=1.0,
                    )
                    rstd_t = statp.tile([128, 1], f32, tag="rstd")
                    nc.vector.reciprocal(out=rstd_t, in_=std_t)
                    xn_bf = xnpool.tile([128, D], bf16, tag="xn", name=f"xn_{i}")
                    nc.vector.tensor_scalar(
                        out=xn_bf, in0=x_t,
                        scalar1=mv[:, 0:1], scalar2=rstd_t,
                        op0=mybir.AluOpType.subtract, op1=mybir.AluOpType.mult,
                    )
                    xns[i] = xn_bf
                return f

            def tr_item(i):
                def f():
                    pst = projps.tile([128, 512], f32, tag="proj", name=f"pst_{i}")
                    pstb = pst[:, :].bitcast(bf16)  # [128, 1024] view
                    xn_bf = xns[i]
                    for c in range(NCH):
                        nc.tensor.transpose(
                            pstb[:, c * 128 : (c + 1) * 128],
                            xn_bf[:, c * 128 : (c + 1) * 128],
                            identb,
                        )
                    nc.vector.tensor_copy(
                        xnT[:, :, i * 128 : (i + 1) * 128],
                        pstb.rearrange("p (c x) -> p c x", x=128),
                    )
                return f

            def v_item(i):
                def f():
                    psv = projps.tile([128, 512], f32, tag="proj", name=f"psv_{i}")
                    for c in range(NCH):
                        nc.tensor.matmul(
                            psv,
                            xnT[:, c, i * 128 : (i + 1) * 128],
                            wv_t[:, c, :],
                            start=(c == 0),
                            stop=(c == NCH - 1 and not has_v_bias),
                        )
                    if has_v_bias:
                        nc.tensor.matmul(psv, vones_t, bv1_t, start=False, stop=True)
                    psv3 = psv.rearrange("p (h v) -> p h v", v=HD)
                    nc.vector.tensor_copy(vh[:, i, :, :], psv3)
                    nc.vector.tensor_tensor(
                        out=vl[:, i, :, :], in0=psv3, in1=vh[:, i, :, :],
                        op=mybir.AluOpType.subtract,
                    )
                    if i < 4:
                        nc.vector.tensor_copy(vb16[:, i, :, :], psv3)
                return f

            def qk_item(sb, fb):
                def f():
                    t, p = fb // 4, fb % 4
                    ps = projps.tile([128, 512], f32, tag="proj",
                                     name=f"psqk_{sb}_{fb}")
                    for c in range(NCH):
                        nc.tensor.matmul(
                            ps,
                            wqk_t[:, c, fb * 128 : (fb + 1) * 128],
                            xnT[:, c, sb * 512 : (sb + 1) * 512],
                            start=(c == 0),
                            stop=(c == NCH - 1),
                        )
                    dest = qT if t == 0 else kT
                    dsl = dest[:, p, sb * 512 : (sb + 1) * 512]
                    if has_qk_bias:
                        nc.scalar.activation(
                            out=dsl, in_=ps,
                            func=mybir.ActivationFunctionType.Identity,
                            bias=bqk_t[:, t, p : p + 1], scale=1.0,
                        )
                    else:
                        nc.scalar.copy(dsl, ps)
                return f

            def op_item(i):
                def f():
                    y_t = ypool.tile([128, D], f32, tag="y", name=f"y_{i}")
                    for nh in range(2):
                        pso = projps.tile([128, 512], f32, tag="proj",
                                          name=f"pso_{i}_{nh}")
                        for c in range(4):
                            nc.tensor.matmul(
                                pso,
                                ytall[:, c, i * 128 : (i + 1) * 128],
                                wout_t[:, c, nh * 512 : (nh + 1) * 512],
                                start=(c == 0),
                                stop=(c == 3),
                            )
                        nc.vector.tensor_copy(
                            y_t[:, nh * 512 : (nh + 1) * 512], pso
                        )
                        nc.sync.dma_start(
                            out=out_d[i * 128 : (i + 1) * 128,
                                      nh * 512 : (nh + 1) * 512],
                            in_=y_t[:, nh * 512 : (nh + 1) * 512],
                        )
                return f

            proj_q = deque()

            def drain(n):
                for _ in range(n):
                    if not proj_q:
                        return
                    proj_q.popleft()()

            # ================= attention =================
            # Software-pipelined: PV/epilogue closures are deferred one pair
            # behind the score/exp emission (PE queues are in-order, so an
            # eagerly-emitted PV would stall the PE on its own pair's
            # exp+mask latency instead of running the next pair's scores).
            # deferred closures, gated by a per-pair sequence number so the
            # epilogue's DMA broadcast latency is hidden by at least one
            # pair of other work before the DVE mult is emitted
            pending = deque()   # (ready_seq, closure)
            seq = [0]
            drain_ctr = [0]

            NOPIPE = bool(int(os.environ.get("KDBG_NOPIPE", "0")))

            def pump(force=False):
                while pending and (force or NOPIPE or (len(pending) >= 3
                                             and pending[0][0] <= seq[0])):
                    pending.popleft()[1]()

            def attention(sb, p, hf):
                rows = slice(hf * HD, (hf + 1) * HD)
                h = p * 2 + hf
                q0g = sb * 512
                npairs = 2 * sb + 2
                ytp = ytps.tile([HD, 512], f32, tag="yt", name=f"yt_{sb}_{p}_{hf}")
                smp = smps.tile([1, 512], f32, tag="sm", name=f"sm_{sb}_{p}_{hf}")
                for jp in range(npairs):
                    kind = "F" if jp < 2 * sb else ("A" if jp == 2 * sb else "B")
                    qlo = 256 if kind == "B" else 0
                    st2 = stps.tile([128, 2, 512], f32, tag="st",
                                    name=f"st_{sb}_{p}_{hf}_{jp}")
                    for t2 in range(2):
                        j = 2 * jp + t2
                        # the odd diag tile's first q-chunk is fully masked;
                        # skip computing it (exp output there is zeroed)
                        ql = qlo + 128 if (kind != "F" and t2 == 1) else qlo
                        nc.tensor.matmul(
                            st2[:, t2, ql:512],
                            kT[rows, p, j * 128 : (j + 1) * 128],
                            qT[rows, p, q0g + ql : q0g + 512],
                            start=True, stop=True,
                        )
                    if sb == 0:
                        pt2 = ptbp.tile([128, 2, 512], bf16, tag="ptb",
                                        name=f"pt_{sb}_{p}_{hf}_{jp}")
                    else:
                        pt2 = ptp.tile([128, 2, 512], f8, tag="pt",
                                       name=f"pt_{sb}_{p}_{hf}_{jp}")
                    # exp(s - 2): keeps softmax weights inside fp8e4m3's
                    # normal range (max 240; max masked score here is ~6.5)
                    # without pushing early-row weights into subnormals; the
                    # uniform shift cancels exactly in the normalization
                    nc.scalar.activation(
                        out=pt2[:, :, qlo:512],
                        in_=st2[:, :, qlo:512],
                        func=mybir.ActivationFunctionType.Exp,
                        bias=negc_t, scale=1.0,
                    )
                    if kind in ("A", "B"):
                        # causal masks on the fp8 pt tiles (gpsimd, off-PE):
                        # tile t2=0 diag at chunk qlo; tile t2=1 fully masked
                        # at chunk qlo, diag at chunk qlo+128
                        nc.gpsimd.affine_select(
                            out=pt2[:, 0, qlo : qlo + 128],
                            in_=pt2[:, 0, qlo : qlo + 128],
                            compare_op=mybir.AluOpType.is_ge,
                            fill=0.0, base=0,
                            pattern=[[1, 128]], channel_multiplier=-1,
                        )
                        nc.gpsimd.memset(pt2[:, 1, qlo : qlo + 128], 0.0)
                        nc.gpsimd.affine_select(
                            out=pt2[:, 1, qlo + 128 : qlo + 256],
                            in_=pt2[:, 1, qlo + 128 : qlo + 256],
                            compare_op=mybir.AluOpType.is_ge,
                            fill=0.0, base=0,
                            pattern=[[1, 128]], channel_multiplier=-1,
                        )

                    def pv(jp=jp, kind=kind, pt2=pt2, ytp=ytp, smp=smp, h=h,
                           sb=sb, npairs=npairs):
                        chunks = (1,) if kind == "B" else (0, 1)
                        for ch in chunks:
                            q0 = 256 * ch
                            # start=True zeroes the whole 2KB PSUM bank row
                            # (zero-region granularity), so only the very
                            # first matmul on each bank may carry it; chunk
                            # 1 inherits the pending-zero marks from it
                            first = jp == 0 and ch == 0
                            last = jp == (2 * sb if ch == 0 else npairs - 1)
                            if sb == 0:
                                for t2 in range(2):
                                    nc.tensor.matmul(
                                        ytp[:, q0 : q0 + 256],
                                        vb16[:, 2 * jp + t2, h, :],
                                        pt2[:, t2, q0 : q0 + 256],
                                        start=(first and t2 == 0),
                                        stop=(last and t2 == 1),
                                    )
                                    nc.tensor.matmul(
                                        smp[:, q0 : q0 + 256],
                                        ones_bf[:, 0:1],
                                        pt2[:, t2, q0 : q0 + 256],
                                        start=(first and t2 == 0),
                                        stop=(last and t2 == 1),
                                    )
                                continue
                            nc.tensor.matmul(
                                ytp[:, q0 : q0 + 256],
                                vh[:, 2 * jp : 2 * jp + 2, h, :],
                                pt2[:, :, q0 : q0 + 256],
                                start=first, stop=False, perf_mode=DR,
                            )
                            nc.tensor.matmul(
                                ytp[:, q0 : q0 + 256],
                                vl[:, 2 * jp : 2 * jp + 2, h, :],
                                pt2[:, :, q0 : q0 + 256],
                                start=False, stop=last, perf_mode=DR,
                            )
                            nc.tensor.matmul(
                                smp[:, q0 : q0 + 256],
                                ones8[:, :, 0:1],
                                pt2[:, :, q0 : q0 + 256],
                                start=first, stop=last, perf_mode=DR,
                            )

                    seq[0] += 1
                    pending.append((seq[0] + 1, pv))
                    pump()
                    if sb == NQS - 1:
                        # ration the remaining proj work across the long
                        # final superblock (the PE deficit there is small)
                        ctr = drain_ctr[0] = drain_ctr[0] + 1
                        if ctr % 12 == 0:
                            drain(1)
                    else:
                        drain(1)

                # softmax normalization epilogue: 1/sums row -> DRAM ->
                # partition-broadcast DMA -> DVE multiply. Staged across the
                # pending pipeline (the mult is emitted >= 2 pairs after the
                # DMAs are triggered) so the broadcast latency never parks at
                # the head of the DVE in-order queue.
                sinv = sip.tile([1, 512], mybir.dt.float32r, tag="sinv",
                                name=f"sinv_{sb}_{p}_{hf}")
                binv = bip.tile([HD, 512], f32, tag="binv",
                                name=f"binv_{sb}_{p}_{hf}")

                def epi_bcast(sinv=sinv, binv=binv, smp=smp):
                    # 1/sums (fp22: 6e-5 relative, inside the error budget),
                    # broadcast across 64 hd partitions via a rank-1 PE
                    # matmul, then evacuated to SBUF for the mult (a
                    # TensorTensor may read only one PSUM operand)
                    with nc.allow_low_precision(reason="fp22 softmax denom"):
                        nc.vector.reciprocal(out=sinv, in_=smp)
                    bps0 = projps.tile([128, 512], f32, tag="proj",
                                       name="binvps")
                    nc.tensor.matmul(bps0[0:HD, :], ones64, sinv,
                                     start=True, stop=True)
                    nc.vector.tensor_copy(binv, bps0[0:HD, :])

                is_last_group = p == 3 and hf == 1

                def epi_mult(ytp=ytp, binv=binv, rows=rows, p=p, q0g=q0g,
                             sb=sb, last_g=is_last_group):
                    nc.vector.tensor_tensor(
                        out=ytall[rows, p, q0g : q0g + 512],
                        in0=ytp, in1=binv, op=mybir.AluOpType.mult,
                    )
                    if last_g:
                        # ytall for this superblock is complete only now, so
                        # the out-proj items must enter the work queue here
                        # (an earlier push could emit their ytall reads
                        # before these writes exist = missing dependency)
                        for i in range(4 * sb, 4 * (sb + 1)):
                            proj_q.append(op_item(i))

                pending.append((seq[0] + 1, epi_bcast))
                pending.append((seq[0] + 2, epi_mult))

            # ================= schedule =================
            # lead-in: x DMAs first, then weights, then transposes/V/QK;
            # sb=0 attention for pair p starts right after its q/k proj
            for i in range(4):
                ln_item(i)()
            wv_t_dma()
            wqk_t_dma()
            for i in range(4):
                tr_item(i)()
                v_item(i)()
            wout_t_dma()

            for sb in range(NQS):
                # leftovers in the queue are prerequisites of THIS
                # superblock's attention (its qk/phaseA items) - they must
                # be emitted before the score/PV reads that consume them
                drain(len(proj_q))
                if sb < NQS - 1:
                    for i in range(4 * (sb + 1), 4 * (sb + 2)):
                        proj_q.append(ln_item(i))
                        proj_q.append(tr_item(i))
                        proj_q.append(v_item(i))
                    for fb in (0, 4, 1, 5, 2, 6, 3, 7):
                        proj_q.append(qk_item(sb + 1, fb))
                for p in range(4):
                    if sb == 0:
                        qk_item(0, p)()
                        qk_item(0, p + 4)()
                    for hf in range(2):
                        attention(sb, p, hf)
                if sb == NQS - 1:
                    pump(force=True)
            drain(len(proj_q))
            if dbg:
                nc.sync.dma_start(out=ytall_d[:, :, :], in_=ytall)
                nc.sync.dma_start(out=qT_d[:, :, :], in_=qT)
                nc.sync.dma_start(out=kT_d[:, :, :], in_=kT)
                nc.sync.dma_start(out=vh_d[:, :, :, :], in_=vh)
                nc.sync.dma_start(out=vl_d[:, :, :, :], in_=vl)

    nc.finalize()
    return nc


def _prep_core_inputs(x, ln_scale, ln_bias, w_qkv, b_qkv, w_out):
    """Host-side folding + per-core input maps. Returns (in_maps, flags)."""
    scale = np.float32(HD ** -0.5)
    # qkv = xn@W + b_qkv, xn = z*ln_scale + ln_bias
    #   =>  z @ (ln_scale*W) + (ln_bias@W + b_qkv)
    b_eff = b_qkv + np.einsum(
        "d,dhf->hf", ln_bias.astype(np.float64), w_qkv.astype(np.float64)
    ).astype(np.float32)
    w_eff = ln_scale[:, None, None] * w_qkv
    wq = w_eff[:, :, 0:64] * scale
    wk = w_eff[:, :, 64:128]
    wv_ = w_eff[:, :, 128:192]
    bq = b_eff[:, 0:64] * scale
    bk = b_eff[:, 64:128]
    bv = b_eff[:, 128:192]

    has_qk_bias = bool(np.any(bq) or np.any(bk))
    has_v_bias = bool(np.any(bv))

    in_maps = []
    for core in range(8):
        b, g = core // 2, core % 2
        hsel = slice(g * HL, (g + 1) * HL)
        # feature order: f = t*512 + p*128 + hf*64 + hd  (local head = 2p+hf)
        qfeat = wq[:, hsel].reshape(D, 512)
        kfeat = wk[:, hsel].reshape(D, 512)
        wqk = np.concatenate([qfeat, kfeat], axis=1)  # [D, 1024]
        wqk = np.ascontiguousarray(
            wqk.reshape(NCH, 128, 1024).transpose(1, 0, 2)
        ).astype(ml_dtypes.bfloat16)
        wvg = np.ascontiguousarray(
            wv_[:, hsel].reshape(D, 512).reshape(NCH, 128, 512).transpose(1, 0, 2)
        ).astype(ml_dtypes.bfloat16)
        wog = np.ascontiguousarray(
            w_out[g * 512 : (g + 1) * 512, :].reshape(4, 128, 1024).transpose(1, 0, 2)
        ).astype(ml_dtypes.bfloat16)
        m = {
            "x": np.ascontiguousarray(x[b]),
            "wqk": wqk,
            "wv": wvg,
            "wout": wog,
        }
        if has_qk_bias:
            bq_p = bq[hsel].reshape(4, 2, 64)  # [p, hf, hd]
            bk_p = bk[hsel].reshape(4, 2, 64)
            bqk = np.stack(
                [
                    bq_p.transpose(1, 2, 0).reshape(128, 4),
                    bk_p.transpose(1, 2, 0).reshape(128, 4),
                ],
                axis=1,
            )  # [128, 2, 4]
            m["bqk"] = np.ascontiguousarray(bqk)
        if has_v_bias:
            m["vones"] = np.ones((1, 128), ml_dtypes.bfloat16)
            m["bv1"] = np.ascontiguousarray(
                bv[hsel].reshape(1, 512)
            ).astype(ml_dtypes.bfloat16)
        in_maps.append(m)
    return in_maps, (has_qk_bias, has_v_bias)


def kernel(x, mask, ln_scale, ln_bias, w_qkv, b_qkv, w_out, b_out, **run_kwargs):
    x = np.asarray(x, np.float32)
    ln_scale = np.asarray(ln_scale, np.float32)
    ln_bias = np.asarray(ln_bias, np.float32)
    w_qkv = np.asarray(w_qkv, np.float32)
    b_qkv = np.asarray(b_qkv, np.float32)
    w_out = np.asarray(w_out, np.float32)
    b_out = np.asarray(b_out, np.float32)
    in_maps, flags = _prep_core_inputs(x, ln_scale, ln_bias, w_qkv, b_qkv, w_out)
    key = ("nc", flags)
    if key not in _cache:
        _cache[key] = build_program(*flags)
    nc = _cache[key]
    _cache["nc"] = nc
    res = run_bass_kernel_spmd(nc, in_maps, list(range(8)), **run_kwargs)
    _cache["last_result"] = res
    out = np.empty((B, S, D), np.float32)
    for b in range(B):
        out[b] = res.results[2 * b]["out"] + res.results[2 * b + 1]["out"]
    out += np.asarray(b_out)[None, None, :]
    return out
